# revision 5
# baseline (speedup 1.0000x reference)
"""Trainium2 Bass kernel for InvariantMessage GNN message passing.

out[e, :] = (MLP(s_j)[nbrs[e,1]]) * ((rbf(dist[e]) @ W_rbf + b_rbf) * env(dist[e]))

The axon tunnel (~60-100 MB/s up, ~30-50 MB/s down) dominates wall time —
measured device execution is ~0.1 s while the baseline call took ~28 s — so
this version minimizes bytes on the wire rather than device cycles:

  - fp16 everywhere on device (weights, node features, inv table, rbf
    matmul). HW-validated: fp16 matmul is exact, and a single 50176-row
    fp16 table supports indirect-DMA gathers with int32 row indices up to
    50175 (the old invA/invB 32768-split was only needed for 512B f32 rows).
  - the inv table is Internal DRAM scratch - nothing uploaded for it.
  - node features are sharded 8-ways (1.6 MB/core fp16); each core runs the
    MLP on its 6272-node slice and an on-device HBM AllGather (replica
    group [0..7]) assembles the full 50176-row table on every core.
  - dist/idx are uploaded raw per edge shard (0.4 MB each per core) in a
    host-pretransposed [NCH, 128, 24] layout so all device DMAs are
    contiguous; the [21, e] rbf lhsT is built on device: sin in an
    edge-partition layout [128e, 20] via fp32 magic-number range reduction,
    pre-scaled by env(d)/d, then one TensorE transpose per 128 edges.
  - output is int8 with a per-edge fp32 scale (f32->int8 cast is
    round-to-nearest with saturation on HW); the host does a single-pass
    strided dequant-multiply into the final array. 13.4 MB/core down
    instead of 53.5 MB/core.

Per-edge math on device (col = 128 edges):
  u = coef_k * d            (coef_k = (k+1)/10, i.e. k pi d / 5 / 2pi)
  v = u - round(u)          (fp32 magic-number rounding)
  sv = [sin(2 pi v) k<20 ; d] * (env(d)/d)      # [128e, 21] fp16
  lhsT = sv^T via TensorE transpose             # [21, 128e]
  ws = lhsT^T @ [W_rbf; b_rbf]                  # PSUM f32 [128e, 128f]
  m = ws * phi_gathered                         # f32
  q = int8(m * 127/absmax_row), scale_out = absmax_row/127

Edges are sharded 100000/core, padded to 33 chunks of 3072 (pad slots gather
row 0 with d=1 and are dropped on the host).
"""
import sys

sys.path.insert(0, "/opt/trn_rl_repo")

import numpy as np

# Persistent XLA compilation cache: run_bass_kernel_spmd rebuilds its jit
# closures every call, so the in-memory jit cache never hits. A disk cache
# keyed on HLO fingerprint skips the XLA+neuronxcc recompile both within a
# process (saves ~0.6 s/call) and across processes (first call ~7 s instead
# of 30-190 s). Harmless no-op if the plugin doesn't support serialization.
try:
    import jax as _jax
    _jax.config.update("jax_compilation_cache_dir", "/tmp/jax_comp_cache")
    for _k, _v in (("jax_persistent_cache_min_compile_time_secs", 0),
                   ("jax_persistent_cache_min_entry_size_bytes", -1)):
        try:
            _jax.config.update(_k, _v)
        except Exception:
            pass
except Exception:
    pass

import concourse.tile as tile
from concourse import bass, bacc, mybir
from concourse.bass_utils import run_bass_kernel_spmd

F32 = mybir.dt.float32
F16 = mybir.dt.float16
I32 = mybir.dt.int32
I8 = mybir.dt.int8

N_CORES = 8
N_ATOMS = 50000
N_EDGES = 800000
D = 128
NB = 20
CUTOFF = 5.0
MAGIC = float(np.float32(1.5 * 2**23))

NODE_PAD = 50176                  # 98 * 512
NSH = NODE_PAD // N_CORES         # 6272 nodes per core (12×512 + 128)
NODE_CHUNKS = [512] * 12 + [128]  # column chunking of the per-core slice
E_CORE = N_EDGES // N_CORES       # 100000
GCH = 3072                        # edges per chunk
SC = GCH // D                     # 24 cols of 128 edges per chunk
NCH = (E_CORE + GCH - 1) // GCH   # 33
E_PAD = NCH * GCH                 # 101376


def build_nc():
    nc = bacc.Bacc(None, target_bir_lowering=False)

    s_jT = nc.dram_tensor("s_jT", [D, NSH], F16, kind="ExternalInput")
    W1h = nc.dram_tensor("W1h", [D, D], F16, kind="ExternalInput")
    W2h = nc.dram_tensor("W2h", [D, D], F16, kind="ExternalInput")
    b1c = nc.dram_tensor("b1c", [D, 1], F32, kind="ExternalInput")
    b2c = nc.dram_tensor("b2c", [D, 1], F32, kind="ExternalInput")
    wextb = nc.dram_tensor("wextb", [D, D], F16, kind="ExternalInput")
    identh = nc.dram_tensor("identh", [D, D], F16, kind="ExternalInput")
    coefC = nc.dram_tensor("coefC", [D, 32], F32, kind="ExternalInput")
    nhp = nc.dram_tensor("nhp", [D, 1], F32, kind="ExternalInput")
    distL = nc.dram_tensor("distL", [NCH, D, SC], F32, kind="ExternalInput")
    idxL = nc.dram_tensor("idxL", [NCH, D, SC], I32, kind="ExternalInput")
    qout = nc.dram_tensor("qout", [NCH, D, SC, D], I8, kind="ExternalOutput")
    sout = nc.dram_tensor("sout", [NCH, D, SC], F16, kind="ExternalOutput")

    invS = nc.dram_tensor("invS", [NSH, D], F16)       # this core's inv slice
    invT = nc.dram_tensor("invT", [NODE_PAD, D], F16)  # AllGather of invS

    with tile.TileContext(nc) as tc:
        with tc.tile_pool(name="const", bufs=1) as cpool, \
             tc.tile_pool(name="mlp", bufs=3) as mpool, \
             tc.tile_pool(name="mlppsum", bufs=1, space="PSUM") as mpsum, \
             tc.tile_pool(name="tpsum", bufs=2, space="PSUM") as tpsum, \
             tc.tile_pool(name="edge", bufs=3) as epool, \
             tc.tile_pool(name="big", bufs=2) as bpool, \
             tc.tile_pool(name="wpsum", bufs=2, space="PSUM") as wpsum:

            w1_sb = cpool.tile([D, D], F16)
            nc.sync.dma_start(out=w1_sb[:], in_=W1h[:])
            w2_sb = cpool.tile([D, D], F16)
            nc.sync.dma_start(out=w2_sb[:], in_=W2h[:])
            b1_sb = cpool.tile([D, 1], F32)
            nc.sync.dma_start(out=b1_sb[:], in_=b1c[:])
            b2_sb = cpool.tile([D, 1], F32)
            nc.sync.dma_start(out=b2_sb[:], in_=b2c[:])
            wext_sb = cpool.tile([D, D], F16)
            nc.sync.dma_start(out=wext_sb[:], in_=wextb[:])
            id_sb = cpool.tile([D, D], F16)
            nc.sync.dma_start(out=id_sb[:], in_=identh[:])
            coef_sb = cpool.tile([D, 32], F32)
            nc.sync.dma_start(out=coef_sb[:], in_=coefC[:])
            nhp_sb = cpool.tile([D, 1], F32)
            nc.sync.dma_start(out=nhp_sb[:], in_=nhp[:])

            # ---- Phase 1: node MLP for this core's 6272-node slice ----
            n0 = 0
            for ncols in NODE_CHUNKS:
                s_t = mpool.tile([D, 512], F16, tag="s")
                nc.sync.dma_start(out=s_t[:, 0:ncols],
                                  in_=s_jT[:, n0:n0 + ncols])
                ph = mpsum.tile([D, 512], F32, tag="ph")
                nc.tensor.matmul(out=ph[:, 0:ncols], lhsT=w1_sb[:],
                                 rhs=s_t[:, 0:ncols], start=True, stop=True)
                h_t = mpool.tile([D, 512], F16, tag="h")
                nc.scalar.activation(out=h_t[:, 0:ncols], in_=ph[:, 0:ncols],
                                     func=mybir.ActivationFunctionType.Silu,
                                     bias=b1_sb[:, 0:1], scale=1.0)
                pi = mpsum.tile([D, 512], F32, tag="pi")
                nc.tensor.matmul(out=pi[:, 0:ncols], lhsT=w2_sb[:],
                                 rhs=h_t[:, 0:ncols], start=True, stop=True)
                iv = mpool.tile([D, 512], F16, tag="iv")
                nc.vector.tensor_scalar_add(out=iv[:, 0:ncols],
                                            in0=pi[:, 0:ncols],
                                            scalar1=b2_sb[:, 0:1])
                for j in range(ncols // D):
                    pt = tpsum.tile([D, D], F16, tag="pt")
                    nc.tensor.transpose(out=pt[:], in_=iv[:, j * D:(j + 1) * D],
                                        identity=id_sb[:])
                    ot = mpool.tile([D, D], F16, tag="ot")
                    nc.scalar.copy(out=ot[:], in_=pt[:])
                    m0 = n0 + j * D
                    nc.sync.dma_start(out=invS[m0:m0 + D, :], in_=ot[:])
                n0 += ncols

            # ---- AllGather the 8 slices into the full inv table ----
            nc.gpsimd.collective_compute(
                "AllGather", mybir.AluOpType.bypass,
                replica_groups=[list(range(N_CORES))],
                ins=[invS[:, :]], outs=[invT[:, :]])

            # ---- Phase 2: edges ----
            for g in range(NCH):
                ix = epool.tile([D, SC], I32, tag="ix")
                nc.sync.dma_start(out=ix[:], in_=idxL[g])
                dt = epool.tile([D, SC], F32, tag="dt")
                nc.sync.dma_start(out=dt[:], in_=distL[g])

                rd = epool.tile([D, SC], F32, tag="rd")
                nc.vector.reciprocal(out=rd[:], in_=dt[:])
                cs = epool.tile([D, SC], F32, tag="cs")
                nc.scalar.activation(out=cs[:], in_=dt[:],
                                     func=mybir.ActivationFunctionType.Sin,
                                     scale=float(np.pi / CUTOFF),
                                     bias=nhp_sb[:, 0:1])
                env = epool.tile([D, SC], F32, tag="env")
                nc.vector.tensor_scalar(out=env[:], in0=cs[:],
                                        scalar1=-0.5, scalar2=0.5,
                                        op0=mybir.AluOpType.mult,
                                        op1=mybir.AluOpType.add)
                scl = epool.tile([D, SC], F32, tag="scl")
                nc.vector.tensor_tensor(out=scl[:], in0=env[:], in1=rd[:],
                                        op=mybir.AluOpType.mult)

                phig = bpool.tile([D, SC, D], F16, tag="phi")
                for s in range(SC):
                    nc.gpsimd.indirect_dma_start(
                        out=phig[:, s, :], out_offset=None, in_=invT[:],
                        in_offset=bass.IndirectOffsetOnAxis(
                            ap=ix[:, s:s + 1], axis=0))

                msb = bpool.tile([D, SC, D], F32, tag="msb")
                amax = epool.tile([D, SC], F32, tag="amax")
                for s3 in range(0, SC, 3):
                    u3 = epool.tile([D, 3, 32], F32, tag="u")
                    for j in range(3):
                        nc.scalar.activation(
                            out=u3[:, j, :], in_=coef_sb[:],
                            func=mybir.ActivationFunctionType.Copy,
                            scale=dt[:, s3 + j:s3 + j + 1])
                    kf3 = epool.tile([D, 3, 32], F32, tag="kf")
                    nc.vector.tensor_scalar(out=kf3[:], in0=u3[:],
                                            scalar1=MAGIC, scalar2=MAGIC,
                                            op0=mybir.AluOpType.add,
                                            op1=mybir.AluOpType.subtract)
                    v3t = epool.tile([D, 3, 32], F32, tag="v")
                    nc.vector.tensor_tensor(out=v3t[:], in0=u3[:], in1=kf3[:],
                                            op=mybir.AluOpType.subtract)
                    sv = epool.tile([D, 3, 32], F16, tag="sv")
                    svs = epool.tile([D, 3, 32], F16, tag="svs")
                    for j in range(3):
                        # cols 20..31 have coef 0 -> sin gives exact zeros,
                        # then col 20 is overwritten with raw d
                        nc.scalar.activation(
                            out=sv[:, j, :], in_=v3t[:, j, :],
                            func=mybir.ActivationFunctionType.Sin,
                            scale=float(2 * np.pi))
                        nc.scalar.copy(out=sv[:, j, NB:NB + 1],
                                       in_=dt[:, s3 + j:s3 + j + 1])
                        nc.vector.tensor_scalar_mul(
                            out=svs[:, j, :], in0=sv[:, j, :],
                            scalar1=scl[:, s3 + j:s3 + j + 1])
                    pt2 = tpsum.tile([96, D], F16, tag="pt2")
                    nc.tensor.transpose(out=pt2[:], in_=svs[:],
                                        identity=id_sb[:])
                    lt = epool.tile([96, D], F16, tag="lt")
                    nc.scalar.copy(out=lt[:], in_=pt2[:])
                    for j in range(3):
                        s = s3 + j
                        pw = wpsum.tile([D, D], F32, tag="pw")
                        nc.tensor.matmul(
                            out=pw[:], lhsT=lt[32 * j:32 * j + NB + 1, :],
                            rhs=wext_sb[32 * j:32 * j + NB + 1, :],
                            start=True, stop=True)
                        nc.vector.tensor_tensor(out=msb[:, s, :], in0=pw[:],
                                                in1=phig[:, s, :],
                                                op=mybir.AluOpType.mult)
                        nc.vector.tensor_reduce(out=amax[:, s:s + 1],
                                                in_=msb[:, s, :],
                                                axis=mybir.AxisListType.X,
                                                op=mybir.AluOpType.max,
                                                apply_absolute_value=True)

                amc = epool.tile([D, SC], F32, tag="amc")
                nc.vector.tensor_scalar_max(out=amc[:], in0=amax[:],
                                            scalar1=1e-20)
                sct = epool.tile([D, SC], F32, tag="sct")
                nc.vector.tensor_scalar_mul(out=sct[:], in0=amc[:],
                                            scalar1=float(1.0 / 127.0))
                sct16 = epool.tile([D, SC], F16, tag="sct16")
                nc.scalar.copy(out=sct16[:], in_=sct[:])
                nc.sync.dma_start(out=sout[g], in_=sct16[:])
                rst = epool.tile([D, SC], F32, tag="rst")
                nc.vector.reciprocal(out=rst[:], in_=sct[:])

                qsb = bpool.tile([D, SC, D], I8, tag="qsb")
                for s in range(SC):
                    nc.scalar.activation(out=qsb[:, s, :], in_=msb[:, s, :],
                                         func=mybir.ActivationFunctionType.Copy,
                                         scale=rst[:, s:s + 1])
                nc.sync.dma_start(out=qout[g], in_=qsb[:])
    nc.finalize()
    return nc


_NC_CACHE = {}


def kernel(s_j, dist, nbrs, W1, b1, W2, b2, W_rbf, b_rbf):
    s_j = np.asarray(s_j, dtype=np.float32)
    dist = np.asarray(dist, dtype=np.float32)
    idx_all = np.asarray(nbrs)[:, 1].astype(np.int32)

    s_jT_full = np.zeros((D, NODE_PAD), dtype=np.float16)
    s_jT_full[:, :N_ATOMS] = s_j.T
    wextb = np.zeros((D, D), dtype=np.float16)
    for qj in range(3):
        wextb[32 * qj:32 * qj + NB] = np.asarray(W_rbf, np.float32)
        wextb[32 * qj + NB] = np.asarray(b_rbf, np.float32)
    coefC = np.zeros((D, 32), dtype=np.float32)
    coefC[:, :NB] = np.arange(1, NB + 1, dtype=np.float32) / 10.0
    common = {
        "W1h": np.asarray(W1, np.float32).astype(np.float16),
        "W2h": np.asarray(W2, np.float32).astype(np.float16),
        "b1c": np.asarray(b1, np.float32).reshape(D, 1),
        "b2c": np.asarray(b2, np.float32).reshape(D, 1),
        "wextb": wextb,
        "identh": np.eye(D, dtype=np.float16),
        "coefC": coefC,
        "nhp": np.full((D, 1), -np.pi / 2, dtype=np.float32),
    }

    in_maps = []
    for c in range(N_CORES):
        sl = slice(c * E_CORE, (c + 1) * E_CORE)
        idx_pad = np.zeros(E_PAD, dtype=np.int32)
        idx_pad[:E_CORE] = idx_all[sl]
        dist_pad = np.ones(E_PAD, dtype=np.float32)
        dist_pad[:E_CORE] = dist[sl]
        idxL = np.ascontiguousarray(
            idx_pad.reshape(NCH, SC, D).transpose(0, 2, 1))
        distL = np.ascontiguousarray(
            dist_pad.reshape(NCH, SC, D).transpose(0, 2, 1))
        s_jT = np.ascontiguousarray(s_jT_full[:, c * NSH:(c + 1) * NSH])
        in_maps.append(dict(common, s_jT=s_jT, distL=distL, idxL=idxL))

    if "nc" not in _NC_CACHE:
        _NC_CACHE["nc"] = build_nc()
    nc = _NC_CACHE["nc"]

    res = run_bass_kernel_spmd(nc, in_maps, list(range(N_CORES)))
    out = np.empty((N_EDGES, D), dtype=np.float32)
    nfull = E_CORE // GCH                     # 32 full chunks per core
    rem = E_CORE - nfull * GCH                # 1696 edges in the tail chunk
    rs = rem // D                             # 13 full cols
    r2 = rem - rs * D                         # 32 edges in the last col
    for c in range(N_CORES):
        q = res.results[c]["qout"]            # [NCH, D, SC, D] int8
        sc = res.results[c]["sout"].astype(np.float32)   # [NCH, D, SC]
        o = out[c * E_CORE:(c + 1) * E_CORE]
        # single-pass dequant straight into the output (edge-major view)
        np.multiply(q[:nfull].transpose(0, 2, 1, 3),
                    sc[:nfull].transpose(0, 2, 1)[..., None],
                    out=o[:nfull * GCH].reshape(nfull, SC, D, D))
        qt = q[nfull].transpose(1, 0, 2)      # [SC, D, D]
        st = sc[nfull].T                      # [SC, D]
        ot = o[nfull * GCH:]
        np.multiply(qt[:rs], st[:rs, :, None],
                    out=ot[:rs * D].reshape(rs, D, D))
        if r2:
            np.multiply(qt[rs, :r2], st[rs, :r2, None], out=ot[rs * D:])
    return out
